# revision 29
# baseline (speedup 1.0000x reference)
import sys

for _p in ("/opt/trn_rl_repo",):
    if _p not in sys.path:
        sys.path.insert(0, _p)

import numpy as np
import ml_dtypes

import concourse.bass as bass
import concourse.bacc as bacc
import concourse.tile as tile
import concourse.mybir as mybir
from concourse import bass_utils

F32 = mybir.dt.float32
BF16 = mybir.dt.bfloat16
FP8 = mybir.dt.float8e4

NP_BF16 = ml_dtypes.bfloat16
NP_FP8 = ml_dtypes.float8_e4m3

EMBED = 512
MID = 512
FINAL = 1024
GLIMPSES = 2
NOBJ = 2048
NREL = 32768
NCORES = 8
RCH = NREL // NCORES          # 4096 relations per core
NOB = NOBJ // 128             # 16 object partition-blocks
VSCALE = float(2 ** 12)      # fp8 scaling for the abc (v) tables
QSCALE = float(2 ** 12)      # fp8 scaling for the qw tables
CSCALE = float(2 ** 24)      # fp8 scaling for qs (atten-normalized q)
HSCALE = 1.0 / (CSCALE * VSCALE)

_CACHE = {}


def _wn(v, g):
    return (v * (g / np.linalg.norm(v.astype(np.float64)))).astype(np.float32)


def _build():
    """Builds the Bass program once. Returns (nc, input tensor names)."""
    nc = bacc.Bacc(
        "TRN2",
        target_bir_lowering=False,
        debug=False,
        enable_asserts=False,
        num_devices=NCORES,
    )

    # ---- DRAM I/O -------------------------------------------------------
    d = {}
    d["d_g8"] = nc.dram_tensor("g8", [NOBJ, RCH], FP8, kind="ExternalInput")
    d["d_oht"] = nc.dram_tensor("oht", [256, RCH], FP8, kind="ExternalInput")
    d["d_abc"] = nc.dram_tensor("abc", [GLIMPSES, 256, MID], FP8, kind="ExternalInput")
    d["d_ohet"] = nc.dram_tensor("ohet", [256, NOBJ], FP8, kind="ExternalInput")
    d["d_qw"] = nc.dram_tensor("qw", [GLIMPSES, 256, MID], FP8, kind="ExternalInput")
    d["d_k0T"] = nc.dram_tensor("k0T", [MID, MID], BF16, kind="ExternalInput")
    d["d_kb0"] = nc.dram_tensor("kb0", [1, MID], F32, kind="ExternalInput")
    d["d_m0T"] = nc.dram_tensor("m0T", [MID, MID], BF16, kind="ExternalInput")
    d["d_m1T"] = nc.dram_tensor("m1T", [MID, MID], BF16, kind="ExternalInput")
    d["d_u"] = nc.dram_tensor("u", [128, 4], F32, kind="ExternalInput")
    d["d_fc2T"] = nc.dram_tensor("fc2T", [MID, FINAL], BF16, kind="ExternalInput")
    d["d_fc2b"] = nc.dram_tensor("fc2b", [1, FINAL], BF16, kind="ExternalInput")
    d["d_recipC"] = nc.dram_tensor("recipC", [128, NOB], F32, kind="ExternalInput")
    d["d_out"] = nc.dram_tensor("out", [1, FINAL], F32, kind="ExternalOutput")

    with tile.TileContext(nc) as tc:
        _emit(nc, tc, d)

    nc.compile()
    in_names = [
        "g8", "oht", "abc", "ohet", "qw", "k0T", "kb0", "m0T", "m1T", "u",
        "fc2T", "fc2b", "recipC",
    ]
    return nc, in_names


def _emit(nc, tc, d):
    AT = mybir.ActivationFunctionType
    OP = mybir.AluOpType
    DR = mybir.MatmulPerfMode.DoubleRow
    rg = [list(range(NCORES))]

    with (
        tc.tile_pool(name="persist", bufs=1) as pp,
        tc.tile_pool(name="vchp", bufs=1) as vp,
        tc.tile_pool(name="work", bufs=3) as wp,
        tc.tile_pool(name="pt", bufs=6, space="PSUM") as pt,
        tc.tile_pool(name="pw", bufs=2, space="PSUM") as pw,
        tc.tile_pool(name="dram", bufs=1, space="DRAM") as dp,
    ):
        # ---- persistent SBUF tensors & loads (in dependency order) ------
        # Critical prologue tensors: split into row-chunks so they spread
        # across all DMA queues and finish before the g8 bulk steals bandwidth.
        abc3 = []
        for g in range(GLIMPSES):
            t = pp.tile([128, 2, MID], FP8, name=f"abc3_{g}", tag=f"abc3_{g}")
            for i in range(2):
                for h in range(2):
                    nc.sync.dma_start(
                        t[h * 64:(h + 1) * 64, i, :],
                        d["d_abc"][g, i * 128 + h * 64:i * 128 + (h + 1) * 64, :])
            abc3.append(t)

        oht3 = pp.tile([128, 2, RCH], FP8, name="oht3", tag="oht3")
        for i in range(2):
            for h in range(4):
                nc.sync.dma_start(
                    oht3[h * 32:(h + 1) * 32, i, :],
                    d["d_oht"][i * 128 + h * 32:i * 128 + (h + 1) * 32, :])

        ohet3 = pp.tile([128, 2, NOBJ], FP8, name="ohet3", tag="ohet3")
        for i in range(2):
            for h in range(2):
                nc.sync.dma_start(
                    ohet3[h * 64:(h + 1) * 64, i, :],
                    d["d_ohet"][i * 128 + h * 64:i * 128 + (h + 1) * 64, :])

        qw3 = []
        for g in range(GLIMPSES):
            t = pp.tile([128, 2, MID], FP8, name=f"qw3_{g}", tag=f"qw3_{g}")
            nc.sync.dma_start(t[:, 0, :], d["d_qw"][g, 0:128, :])
            nc.sync.dma_start(t[:, 1, :], d["d_qw"][g, 128:256, :])
            qw3.append(t)

        recipC = pp.tile([128, NOB], F32, name="recipC", tag="recipC")
        nc.sync.dma_start(recipC[:], d["d_recipC"][:, :])

        # graph blocks: pairs of 128-row blocks for DoubleRow
        g8p = []
        for b in range(8):
            t = pp.tile([128, 2, RCH], FP8, name=f"g8p{b}", tag=f"g8p{b}")
            nc.sync.dma_start(t[:, 0, :], d["d_g8"][(2 * b) * 128:(2 * b + 1) * 128, :])
            nc.sync.dma_start(t[:, 1, :], d["d_g8"][(2 * b + 1) * 128:(2 * b + 2) * 128, :])
            g8p.append(t)

        # late-use weights
        k0Ts = pp.tile([128, 4 * MID], BF16, name="k0Ts", tag="k0Ts")
        for kb in range(4):
            nc.sync.dma_start(k0Ts[:, kb * MID:(kb + 1) * MID],
                              d["d_k0T"][kb * 128:(kb + 1) * 128, :])
        kb0s = pp.tile([1, MID], F32, name="kb0s", tag="kb0s")
        nc.sync.dma_start(kb0s[:], d["d_kb0"][:, :])
        m0Ts = pp.tile([128, 4 * MID], BF16, name="m0Ts", tag="m0Ts")
        for kb in range(4):
            nc.sync.dma_start(m0Ts[:, kb * MID:(kb + 1) * MID],
                              d["d_m0T"][kb * 128:(kb + 1) * 128, :])
        m1Ts = pp.tile([128, 4 * MID], BF16, name="m1Ts", tag="m1Ts")
        for kb in range(4):
            nc.sync.dma_start(m1Ts[:, kb * MID:(kb + 1) * MID],
                              d["d_m1T"][kb * 128:(kb + 1) * 128, :])
        us = pp.tile([128, 4], F32, name="us", tag="us")
        nc.sync.dma_start(us[:], d["d_u"][:, :])
        fc2Ts = pp.tile([128, 4 * FINAL], BF16, name="fc2Ts", tag="fc2Ts")
        for kb in range(4):
            nc.sync.dma_start(fc2Ts[:, kb * FINAL:(kb + 1) * FINAL],
                              d["d_fc2T"][kb * 128:(kb + 1) * 128, :])
        fc2bs = pp.tile([1, FINAL], BF16, name="fc2bs", tag="fc2bs")
        nc.sync.dma_start(fc2bs[:], d["d_fc2b"][:, :])

        # ---- per-glimpse state ------------------------------------------
        qs3 = [pp.tile([128, 2, MID], FP8, name=f"qs3_{b}", tag=f"qs3_{b}")
               for b in range(8)]
        hpart = [pp.tile([128, 32], F32, name=f"hpart{g}", tag=f"hpart{g}")
                 for g in range(GLIMPSES)]
        hT = [pp.tile([128, 4], F32, name=f"hT{g}", tag=f"hT{g}")
              for g in range(GLIMPSES)]
        hTa = [pp.tile([128, 4], F32, name=f"hTa{g}", tag=f"hTa{g}")
               for g in range(GLIMPSES)]
        hTab = [pp.tile([128, 4], BF16, name=f"hTab{g}", tag=f"hTab{g}")
                for g in range(GLIMPSES)]
        z1bq_sb = pp.tile([1, MID], F32, name="z1bq_sb", tag="z1bq_sb")
        w_sb = pp.tile([128, 4], F32, name="w_sb", tag="w_sb")
        ones1 = pp.tile([1, 1], BF16, name="ones1", tag="ones1")
        nc.vector.memset(ones1[:], 1.0)

        def emit_v(g, mt, rc, engine):
            # vch = relu(abc.T @ oht) chunk [128 m, 512 r]  (scaled by VSCALE)
            vps = pw.tile([128, 512], F32, name=f"vps{g}_{mt}_{rc}", tag="wps")
            nc.tensor.matmul(vps[:],
                             abc3[g][:, :, mt * 128:(mt + 1) * 128],
                             oht3[:, :, rc * 512:(rc + 1) * 512],
                             start=True, stop=True, perf_mode=DR)
            vch = vp.tile([128, 512], BF16, name=f"vch{g}_{mt}_{rc}",
                          tag=f"vch{mt}_{rc}")
            if engine == 0:
                nc.scalar.activation(vch[:], vps[:], AT.Relu)
            else:
                nc.vector.tensor_scalar(vch[:], vps[:], 0.0, None, OP.max)
            return vch

        def emit_qs(g, ot):
            # qs = relu(OHE @ qw) * recipC   (fp8, scaled by CSCALE)
            qps = pw.tile([128, 512], F32, name=f"qps{g}_{ot}", tag="wps")
            nc.tensor.matmul(qps[:],
                             ohet3[:, :, ot * 128:(ot + 1) * 128],
                             qw3[g][:, :, :],
                             start=True, stop=True, perf_mode=DR)
            dst = qs3[ot // 2][:, ot % 2, :]
            if ot % 2 == 0:
                nc.scalar.activation(dst, qps[:], AT.Relu,
                                     scale=recipC[:, ot:ot + 1])
            else:
                nc.vector.tensor_scalar(dst, qps[:], recipC[:, ot:ot + 1],
                                        0.0, OP.mult, OP.max)

        def emit_tps_tile(g, mt, rc, vch):
            # t^T chunk [128 m, 512 r] = sum_ob qs^T @ g8 ; then fused
            # h-partial = sum_r vch * t
            tps = pt.tile([128, 512], F32, name=f"tps{g}_{mt}_{rc}", tag="tps")
            for b in range(8):
                nc.tensor.matmul(
                    tps[:],
                    qs3[b][:, :, mt * 128:(mt + 1) * 128],
                    g8p[b][:, :, rc * 512:(rc + 1) * 512],
                    start=(b == 0), stop=(b == 7), perf_mode=DR)
            scr = wp.tile([128, 512], BF16, name=f"scr{g}_{mt}_{rc}", tag="scr")
            idx = mt * 8 + rc
            nc.vector.tensor_tensor(scr[:], tps[:], vch[:], OP.mult)
            nc.vector.tensor_reduce(hpart[g][:, idx:idx + 1], scr[:],
                                    mybir.AxisListType.X, OP.add)

        def emit_h_reduce(g, mt):
            nc.vector.tensor_reduce(hT[g][:, mt:mt + 1],
                                    hpart[g][:, mt * 8:(mt + 1) * 8],
                                    mybir.AxisListType.X, OP.add)
            nc.vector.tensor_scalar(hT[g][:, mt:mt + 1], hT[g][:, mt:mt + 1],
                                    HSCALE, None, OP.mult)

        def emit_h_allreduce(g):
            h_in = dp.tile([128, 4], F32, name=f"h_in{g}", tag=f"h_in{g}")
            h_out = dp.tile([128, 4], F32, name=f"h_out{g}", tag=f"h_out{g}",
                            addr_space="Shared")
            nc.sync.dma_start(h_in[:], hT[g][:])
            nc.gpsimd.collective_compute(
                "AllReduce", OP.add, replica_groups=rg,
                ins=[h_in[:].opt()], outs=[h_out[:].opt()])
            nc.sync.dma_start(hTa[g][:], h_out[:])

        # ================= schedule =====================================
        # glimpse 0 prologue: v0 + qs0 while g8 streams in
        vch0 = {}
        for mt in range(4):
            for rc in range(8):
                vch0[(mt, rc)] = emit_v(0, mt, rc, (mt * 8 + rc) % 2)
        for ot in range(NOB):
            emit_qs(0, ot)

        # glimpse 0 main loop
        for mt in range(4):
            for rc in range(8):
                emit_tps_tile(0, mt, rc, vch0[(mt, rc)])
            emit_h_reduce(0, mt)
        emit_h_allreduce(0)

        # glimpse-1 v runs in the shadow of the AllReduce wait
        vch1 = {}
        for mt in range(4):
            for rc in range(8):
                vch1[(mt, rc)] = emit_v(1, mt, rc, (mt * 8 + rc) % 2)

        nc.scalar.copy(hTab[0][:], hTa[0][:])

        # z1bq*QSCALE = h0 @ (QSCALE*K0).T + QSCALE*kb0 -> row 160 of qw1
        # (QSCALE folded into k0T/kb0 on the host; one DVE op writes the row)
        zps = pw.tile([1, MID], F32, name="zps", tag="wps")
        for kb in range(4):
            nc.tensor.matmul(zps[:], hTab[0][:, kb:kb + 1],
                             k0Ts[:, kb * MID:(kb + 1) * MID],
                             start=(kb == 0), stop=(kb == 3))
        nc.vector.tensor_tensor(qw3[1][32:33, 1, :], zps[:], kb0s[:], OP.add)

        # glimpse 1: qs1 pipelined into the first tps1 tile's accumulation
        tps10 = pt.tile([128, 512], F32, name="tps10", tag="tps")
        for b in range(8):
            emit_qs(1, 2 * b)
            emit_qs(1, 2 * b + 1)
            nc.tensor.matmul(tps10[:], qs3[b][:, :, 0:128],
                             g8p[b][:, :, 0:512],
                             start=(b == 0), stop=(b == 7), perf_mode=DR)
        scr10 = wp.tile([128, 512], BF16, name="scr10", tag="scr")
        nc.vector.tensor_tensor(scr10[:], tps10[:], vch1[(0, 0)][:], OP.mult)
        nc.vector.tensor_reduce(hpart[1][:, 0:1], scr10[:],
                                mybir.AxisListType.X, OP.add)

        # z0 = M0 @ h0 ; w = u + 2048*z0   (runs under the tps1 window)
        z0ps = pw.tile([128, 4], F32, name="z0ps", tag="wps")
        for jt in range(4):
            for kb in range(4):
                nc.tensor.matmul(
                    z0ps[:, jt:jt + 1],
                    m0Ts[:, kb * MID + jt * 128: kb * MID + (jt + 1) * 128],
                    hTab[0][:, kb:kb + 1], start=(kb == 0), stop=(kb == 3))
        nc.vector.tensor_scalar(w_sb[:], z0ps[:], float(NOBJ), None, OP.mult)
        nc.vector.tensor_tensor(w_sb[:], w_sb[:], us[:], OP.add)

        for mt in range(4):
            for rc in range(8):
                if (mt, rc) == (0, 0):
                    continue
                emit_tps_tile(1, mt, rc, vch1[(mt, rc)])
            emit_h_reduce(1, mt)
        emit_h_allreduce(1)
        nc.scalar.copy(hTab[1][:], hTa[1][:])

        # tail: o1 = relu(u + 2048*(M0@h0 + M1@h1)) = relu(2048*z1t + w)
        z1ps = pw.tile([128, 4], F32, name="z1ps", tag="wps")
        for jt in range(4):
            for kb in range(4):
                nc.tensor.matmul(
                    z1ps[:, jt:jt + 1],
                    m1Ts[:, kb * MID + jt * 128: kb * MID + (jt + 1) * 128],
                    hTab[1][:, kb:kb + 1], start=(kb == 0), stop=(kb == 3))
        o1Tb = pp.tile([128, 4], BF16, name="o1Tb", tag="o1Tb")
        for jt in range(4):
            nc.scalar.activation(o1Tb[:, jt:jt + 1], z1ps[:, jt:jt + 1],
                                 AT.Relu, bias=w_sb[:, jt:jt + 1],
                                 scale=float(NOBJ))

        # fc2: out = relu(o1 @ fc2T + fc2b)   [1, 1024]
        out_sb = pp.tile([1, FINAL], F32, name="out_sb", tag="out_sb")
        for half in range(2):
            ops_ = pw.tile([1, 512], F32, name=f"ops{half}", tag="wps")
            for kb in range(4):
                nc.tensor.matmul(
                    ops_[:], o1Tb[:, kb:kb + 1],
                    fc2Ts[:, kb * FINAL + half * 512: kb * FINAL + half * 512 + 512],
                    start=(kb == 0), stop=False)
            nc.tensor.matmul(
                ops_[:], ones1[:],
                fc2bs[0:1, half * 512:(half + 1) * 512],
                start=False, stop=True)
            nc.scalar.activation(out_sb[0:1, half * 512:(half + 1) * 512],
                                 ops_[:], AT.Relu)
        nc.sync.dma_start(d["d_out"][:, :], out_sb[:])


def _prep_inputs(entities, relations, graph, obj_tab, head_tab, tail_tab, pred_tab,
                 lin_v_v, lin_v_g, lin_v_b, lin_q_v, lin_q_g, lin_q_b,
                 lin_a_v, lin_a_g, lin_a_b, fc1_w, fc1_b, fc2_w, fc2_b):
    ent = np.asarray(entities).astype(np.int64)
    rel = np.asarray(relations).astype(np.int64)
    graph = np.asarray(graph, dtype=np.float32)
    obj_tab = np.asarray(obj_tab, np.float32)
    head_tab = np.asarray(head_tab, np.float32)
    tail_tab = np.asarray(tail_tab, np.float32)
    pred_tab = np.asarray(pred_tab, np.float32)

    fc1_w = np.asarray(fc1_w, np.float32)
    fc1_b = np.asarray(fc1_b, np.float32)

    abc = np.zeros((GLIMPSES, 256, MID), np.float32)
    qw = np.zeros((GLIMPSES, 256, MID), np.float32)
    Wa = [None, None]
    ba = [None, None]
    for g in range(GLIMPSES):
        Wv = _wn(np.asarray(lin_v_v[g], np.float32), float(lin_v_g[g]))
        abc[g, 0:51] = head_tab[:51] @ Wv[:, 0:EMBED].T + np.asarray(lin_v_b[g], np.float32)
        abc[g, 51:102] = tail_tab[:51] @ Wv[:, EMBED:2 * EMBED].T
        abc[g, 102:153] = pred_tab[:51] @ Wv[:, 2 * EMBED:3 * EMBED].T
        Wq = _wn(np.asarray(lin_q_v[g], np.float32), float(lin_q_g[g]))
        qw[g, 0:151] = obj_tab @ Wq.T
        if g == 0:
            qw[0, 0:151] += np.asarray(lin_q_b[0], np.float32)
        Wa[g] = _wn(np.asarray(lin_a_v[g], np.float32), float(lin_a_g[g]))
        ba[g] = np.asarray(lin_a_b[g], np.float32)

    Wq1 = _wn(np.asarray(lin_q_v[1], np.float32), float(lin_q_g[1]))
    # z1bq*QSCALE = h0 @ (QSCALE*(Wq1 @ Wa0)).T + QSCALE*(ba0 @ Wq1.T + bq1)
    k0T = np.ascontiguousarray((Wq1 @ Wa[0]).T) * QSCALE
    kb0 = (ba[0] @ Wq1.T + np.asarray(lin_q_b[1], np.float32)).reshape(1, MID) * QSCALE
    # fc1 @ sg = u + 2048*(M0 @ h0 + M1 @ h1)
    m0T = np.ascontiguousarray((fc1_w @ Wa[0]).T)
    m1T = np.ascontiguousarray((fc1_w @ Wa[1]).T)

    oht = np.zeros((NCORES, 256, RCH), NP_FP8)
    ar = np.arange(RCH)
    for c in range(NCORES):
        rc = rel[c * RCH:(c + 1) * RCH]
        m = np.zeros((256, RCH), np.float32)
        m[rc[:, 0], ar] = 1.0
        m[rc[:, 1] + 51, ar] = 1.0
        m[rc[:, 2] + 102, ar] = 1.0
        oht[c] = m.astype(NP_FP8)

    ohet = np.zeros((256, NOBJ), np.float32)
    ohet[ent, np.arange(NOBJ)] = 1.0
    # ones row at cat 160 (partition 32 of k-subtile 1, ACT-writable):
    # broadcasts the z1bq correction to every object in glimpse 1
    ohet[160, :] = 1.0

    colsum = graph.sum(axis=1, dtype=np.float32) + 1e-9
    recipC = (CSCALE / (colsum * QSCALE)).reshape(NOB, 128).T.copy()

    cnt = np.bincount(ent, minlength=151).astype(np.float32)
    sgq0 = cnt @ obj_tab                       # column sums of q0  [512]
    u = (fc1_w @ sgq0 + float(NOBJ) * (fc1_w @ (ba[0] + ba[1])) + fc1_b)
    u = u.reshape(4, 128).T.copy()

    base = {
        "oht": None,  # per-core
        "abc": (abc * VSCALE).astype(NP_FP8),
        "ohet": ohet.astype(NP_FP8),
        "qw": (qw * QSCALE).astype(NP_FP8),
        "k0T": k0T.astype(NP_BF16),
        "kb0": kb0,
        "m0T": m0T.astype(NP_BF16),
        "m1T": m1T.astype(NP_BF16),
        "u": u,
        "fc2T": np.ascontiguousarray(fc2_w.astype(np.float32).T).astype(NP_BF16),
        "fc2b": np.asarray(fc2_b, np.float32).reshape(1, FINAL).astype(NP_BF16),
        "recipC": recipC,
    }
    in_maps = []
    for c in range(NCORES):
        m = dict(base)
        m["g8"] = np.ascontiguousarray(graph[:, c * RCH:(c + 1) * RCH]).astype(NP_FP8)
        m["oht"] = oht[c]
        in_maps.append(m)
    return in_maps


def kernel(**inputs):
    if "nc" not in _CACHE:
        _CACHE["nc"], _CACHE["in_names"] = _build()
    nc = _CACHE["nc"]
    in_maps = _prep_inputs(**inputs)
    res = bass_utils.run_bass_kernel_spmd(nc, in_maps, core_ids=list(range(NCORES)))
    return np.asarray(res.results[0]["out"], np.float32)


# revision 32
# speedup vs baseline: 1.0847x; 1.0847x over previous
import sys

for _p in ("/opt/trn_rl_repo",):
    if _p not in sys.path:
        sys.path.insert(0, _p)

import numpy as np
import ml_dtypes

import concourse.bass as bass
import concourse.bacc as bacc
import concourse.tile as tile
import concourse.mybir as mybir
from concourse import bass_utils

F32 = mybir.dt.float32
BF16 = mybir.dt.bfloat16
FP8 = mybir.dt.float8e4

NP_BF16 = ml_dtypes.bfloat16
NP_FP8 = ml_dtypes.float8_e4m3

EMBED = 512
MID = 512
FINAL = 1024
GLIMPSES = 2
NOBJ = 2048
NREL = 32768
NCORES = 8
RCH = NREL // NCORES          # 4096 relations per core
NOB = NOBJ // 128             # 16 object partition-blocks
VSCALE = float(2 ** 12)      # fp8 scaling for the abc (v) tables
QSCALE = float(2 ** 12)      # fp8 scaling for the qw tables
CSCALE = float(2 ** 24)      # fp8 scaling for qs (atten-normalized q)
HSCALE = 1.0 / (CSCALE * VSCALE)

_CACHE = {}


def _wn(v, g):
    return (v * (g / np.linalg.norm(v.astype(np.float64)))).astype(np.float32)


def _build():
    """Builds the Bass program once. Returns (nc, input tensor names)."""
    nc = bacc.Bacc(
        "TRN2",
        target_bir_lowering=False,
        debug=False,
        enable_asserts=False,
        num_devices=NCORES,
    )

    # ---- DRAM I/O -------------------------------------------------------
    d = {}
    d["d_g8"] = nc.dram_tensor("g8", [NOBJ, RCH], FP8, kind="ExternalInput")
    d["d_oht"] = nc.dram_tensor("oht", [256, RCH], FP8, kind="ExternalInput")
    d["d_abc"] = nc.dram_tensor("abc", [GLIMPSES, 256, MID], FP8, kind="ExternalInput")
    d["d_ohet"] = nc.dram_tensor("ohet", [256, NOBJ], FP8, kind="ExternalInput")
    d["d_qw"] = nc.dram_tensor("qw", [GLIMPSES, 256, MID], FP8, kind="ExternalInput")
    d["d_k0T"] = nc.dram_tensor("k0T", [MID, MID], BF16, kind="ExternalInput")
    d["d_kb0"] = nc.dram_tensor("kb0", [1, MID], F32, kind="ExternalInput")
    d["d_m0T"] = nc.dram_tensor("m0T", [MID, MID], BF16, kind="ExternalInput")
    d["d_m1T"] = nc.dram_tensor("m1T", [MID, MID], BF16, kind="ExternalInput")
    d["d_u"] = nc.dram_tensor("u", [128, 4], F32, kind="ExternalInput")
    d["d_fc2T"] = nc.dram_tensor("fc2T", [MID, FINAL], BF16, kind="ExternalInput")
    d["d_fc2b"] = nc.dram_tensor("fc2b", [1, FINAL], BF16, kind="ExternalInput")
    d["d_recipC"] = nc.dram_tensor("recipC", [128, NOB], F32, kind="ExternalInput")
    d["d_out"] = nc.dram_tensor("out", [1, FINAL], F32, kind="ExternalOutput")

    with tile.TileContext(nc) as tc:
        _emit(nc, tc, d)

    nc.compile()
    in_names = [
        "g8", "oht", "abc", "ohet", "qw", "k0T", "kb0", "m0T", "m1T", "u",
        "fc2T", "fc2b", "recipC",
    ]
    return nc, in_names


def _emit(nc, tc, d):
    AT = mybir.ActivationFunctionType
    OP = mybir.AluOpType
    DR = mybir.MatmulPerfMode.DoubleRow
    rg = [list(range(NCORES))]

    with (
        tc.tile_pool(name="persist", bufs=1) as pp,
        tc.tile_pool(name="vchp", bufs=1) as vp,
        tc.tile_pool(name="work", bufs=3) as wp,
        tc.tile_pool(name="pt", bufs=6, space="PSUM") as pt,
        tc.tile_pool(name="pw", bufs=2, space="PSUM") as pw,
        tc.tile_pool(name="dram", bufs=1, space="DRAM") as dp,
    ):
        # ---- persistent SBUF tensors & loads (in dependency order) ------
        abc3 = []
        for g in range(GLIMPSES):
            t = pp.tile([128, 2, MID], FP8, name=f"abc3_{g}", tag=f"abc3_{g}")
            nc.sync.dma_start(t[:, 0, :], d["d_abc"][g, 0:128, :])
            nc.sync.dma_start(t[:, 1, :], d["d_abc"][g, 128:256, :])
            abc3.append(t)

        oht3 = pp.tile([128, 2, RCH], FP8, name="oht3", tag="oht3")
        nc.sync.dma_start(oht3[:, 0, :], d["d_oht"][0:128, :])
        nc.sync.dma_start(oht3[:, 1, :], d["d_oht"][128:256, :])

        ohet3 = pp.tile([128, 2, NOBJ], FP8, name="ohet3", tag="ohet3")
        nc.sync.dma_start(ohet3[:, 0, :], d["d_ohet"][0:128, :])
        nc.sync.dma_start(ohet3[:, 1, :], d["d_ohet"][128:256, :])

        qw3 = []
        for g in range(GLIMPSES):
            t = pp.tile([128, 2, MID], FP8, name=f"qw3_{g}", tag=f"qw3_{g}")
            nc.sync.dma_start(t[:, 0, :], d["d_qw"][g, 0:128, :])
            nc.sync.dma_start(t[:, 1, :], d["d_qw"][g, 128:256, :])
            qw3.append(t)

        recipC = pp.tile([128, NOB], F32, name="recipC", tag="recipC")
        nc.sync.dma_start(recipC[:], d["d_recipC"][:, :])

        # graph blocks: pairs of 128-row blocks for DoubleRow
        g8p = []
        for b in range(8):
            t = pp.tile([128, 2, RCH], FP8, name=f"g8p{b}", tag=f"g8p{b}")
            nc.sync.dma_start(t[:, 0, :], d["d_g8"][(2 * b) * 128:(2 * b + 1) * 128, :])
            nc.sync.dma_start(t[:, 1, :], d["d_g8"][(2 * b + 1) * 128:(2 * b + 2) * 128, :])
            g8p.append(t)

        # late-use weights
        k0Ts = pp.tile([128, 4 * MID], BF16, name="k0Ts", tag="k0Ts")
        for kb in range(4):
            nc.sync.dma_start(k0Ts[:, kb * MID:(kb + 1) * MID],
                              d["d_k0T"][kb * 128:(kb + 1) * 128, :])
        kb0s = pp.tile([1, MID], F32, name="kb0s", tag="kb0s")
        nc.sync.dma_start(kb0s[:], d["d_kb0"][:, :])
        m0Ts = pp.tile([128, 4 * MID], BF16, name="m0Ts", tag="m0Ts")
        for kb in range(4):
            nc.sync.dma_start(m0Ts[:, kb * MID:(kb + 1) * MID],
                              d["d_m0T"][kb * 128:(kb + 1) * 128, :])
        m1Ts = pp.tile([128, 4 * MID], BF16, name="m1Ts", tag="m1Ts")
        for kb in range(4):
            nc.sync.dma_start(m1Ts[:, kb * MID:(kb + 1) * MID],
                              d["d_m1T"][kb * 128:(kb + 1) * 128, :])
        us = pp.tile([128, 4], F32, name="us", tag="us")
        nc.sync.dma_start(us[:], d["d_u"][:, :])
        fc2Ts = pp.tile([128, 4 * FINAL], BF16, name="fc2Ts", tag="fc2Ts")
        for kb in range(4):
            nc.sync.dma_start(fc2Ts[:, kb * FINAL:(kb + 1) * FINAL],
                              d["d_fc2T"][kb * 128:(kb + 1) * 128, :])
        fc2bs = pp.tile([1, FINAL], BF16, name="fc2bs", tag="fc2bs")
        nc.sync.dma_start(fc2bs[:], d["d_fc2b"][:, :])

        # ---- per-glimpse state ------------------------------------------
        qs3 = [pp.tile([128, 2, MID], FP8, name=f"qs3_{b}", tag=f"qs3_{b}")
               for b in range(8)]
        hpart = [pp.tile([128, 32], F32, name=f"hpart{g}", tag=f"hpart{g}")
                 for g in range(GLIMPSES)]
        hT = [pp.tile([128, 4], F32, name=f"hT{g}", tag=f"hT{g}")
              for g in range(GLIMPSES)]
        hTa = [pp.tile([128, 4], F32, name=f"hTa{g}", tag=f"hTa{g}")
               for g in range(GLIMPSES)]
        hTab = [pp.tile([128, 4], BF16, name=f"hTab{g}", tag=f"hTab{g}")
                for g in range(GLIMPSES)]
        z1bq_sb = pp.tile([1, MID], F32, name="z1bq_sb", tag="z1bq_sb")
        w_sb = pp.tile([128, 4], F32, name="w_sb", tag="w_sb")
        ones1 = pp.tile([1, 1], BF16, name="ones1", tag="ones1")
        nc.vector.memset(ones1[:], 1.0)

        def emit_v(g, mt, rc, engine):
            # vch = relu(abc.T @ oht) chunk [128 m, 512 r]  (scaled by VSCALE)
            vps = pw.tile([128, 512], F32, name=f"vps{g}_{mt}_{rc}", tag="wps")
            nc.tensor.matmul(vps[:],
                             abc3[g][:, :, mt * 128:(mt + 1) * 128],
                             oht3[:, :, rc * 512:(rc + 1) * 512],
                             start=True, stop=True, perf_mode=DR)
            vch = vp.tile([128, 512], BF16, name=f"vch{g}_{mt}_{rc}",
                          tag=f"vch{mt}_{rc}")
            if engine == 0:
                nc.scalar.activation(vch[:], vps[:], AT.Relu)
            else:
                nc.vector.tensor_scalar(vch[:], vps[:], 0.0, None, OP.max)
            return vch

        def emit_qs(g, ot):
            # qs = relu(OHE @ qw) * recipC   (fp8, scaled by CSCALE)
            qps = pw.tile([128, 512], F32, name=f"qps{g}_{ot}", tag="wps")
            nc.tensor.matmul(qps[:],
                             ohet3[:, :, ot * 128:(ot + 1) * 128],
                             qw3[g][:, :, :],
                             start=True, stop=True, perf_mode=DR)
            dst = qs3[ot // 2][:, ot % 2, :]
            if ot % 2 == 0:
                nc.scalar.activation(dst, qps[:], AT.Relu,
                                     scale=recipC[:, ot:ot + 1])
            else:
                nc.vector.tensor_scalar(dst, qps[:], recipC[:, ot:ot + 1],
                                        0.0, OP.mult, OP.max)

        def emit_tps_tile(g, mt, rc, vch):
            # t^T chunk [128 m, 512 r] = sum_ob qs^T @ g8 ; then fused
            # h-partial = sum_r vch * t
            tps = pt.tile([128, 512], F32, name=f"tps{g}_{mt}_{rc}", tag="tps")
            for b in range(8):
                nc.tensor.matmul(
                    tps[:],
                    qs3[b][:, :, mt * 128:(mt + 1) * 128],
                    g8p[b][:, :, rc * 512:(rc + 1) * 512],
                    start=(b == 0), stop=(b == 7), perf_mode=DR)
            scr = wp.tile([128, 512], BF16, name=f"scr{g}_{mt}_{rc}", tag="scr")
            idx = mt * 8 + rc
            nc.vector.tensor_tensor(scr[:], tps[:], vch[:], OP.mult)
            nc.vector.tensor_reduce(hpart[g][:, idx:idx + 1], scr[:],
                                    mybir.AxisListType.X, OP.add)

        def emit_h_reduce(g, mt):
            nc.vector.tensor_reduce(hT[g][:, mt:mt + 1],
                                    hpart[g][:, mt * 8:(mt + 1) * 8],
                                    mybir.AxisListType.X, OP.add)
            nc.vector.tensor_scalar(hT[g][:, mt:mt + 1], hT[g][:, mt:mt + 1],
                                    HSCALE, None, OP.mult)

        def emit_h_allreduce(g):
            h_in = dp.tile([128, 4], F32, name=f"h_in{g}", tag=f"h_in{g}")
            h_out = dp.tile([128, 4], F32, name=f"h_out{g}", tag=f"h_out{g}",
                            addr_space="Shared")
            nc.sync.dma_start(h_in[:], hT[g][:])
            nc.gpsimd.collective_compute(
                "AllReduce", OP.add, replica_groups=rg,
                ins=[h_in[:].opt()], outs=[h_out[:].opt()])
            nc.sync.dma_start(hTa[g][:], h_out[:])

        # ================= schedule =====================================
        # glimpse 0 prologue: v0 + qs0 while g8 streams in
        vch0 = {}
        for mt in range(4):
            for rc in range(8):
                vch0[(mt, rc)] = emit_v(0, mt, rc, (mt * 8 + rc) % 2)
        for ot in range(NOB):
            emit_qs(0, ot)

        # glimpse 0 main loop
        for mt in range(4):
            for rc in range(8):
                emit_tps_tile(0, mt, rc, vch0[(mt, rc)])
            emit_h_reduce(0, mt)
        emit_h_allreduce(0)

        # glimpse-1 v runs in the shadow of the AllReduce wait
        vch1 = {}
        for mt in range(4):
            for rc in range(8):
                vch1[(mt, rc)] = emit_v(1, mt, rc, (mt * 8 + rc) % 2)

        nc.scalar.copy(hTab[0][:], hTa[0][:])

        # z1bq*QSCALE = h0 @ (QSCALE*K0).T + QSCALE*kb0 -> row 160 of qw1
        # (QSCALE folded into k0T/kb0 on the host; one DVE op writes the row)
        zps = pw.tile([1, MID], F32, name="zps", tag="wps")
        for kb in range(4):
            nc.tensor.matmul(zps[:], hTab[0][:, kb:kb + 1],
                             k0Ts[:, kb * MID:(kb + 1) * MID],
                             start=(kb == 0), stop=(kb == 3))
        nc.vector.tensor_tensor(qw3[1][32:33, 1, :], zps[:], kb0s[:], OP.add)

        # glimpse 1
        for ot in range(NOB):
            emit_qs(1, ot)

        # z0 = M0 @ h0 ; w = u + 2048*z0   (runs under the tps1 window)
        z0ps = pw.tile([128, 4], F32, name="z0ps", tag="wps")
        for jt in range(4):
            for kb in range(4):
                nc.tensor.matmul(
                    z0ps[:, jt:jt + 1],
                    m0Ts[:, kb * MID + jt * 128: kb * MID + (jt + 1) * 128],
                    hTab[0][:, kb:kb + 1], start=(kb == 0), stop=(kb == 3))
        nc.vector.tensor_scalar(w_sb[:], z0ps[:], float(NOBJ), None, OP.mult)
        nc.vector.tensor_tensor(w_sb[:], w_sb[:], us[:], OP.add)

        for mt in range(4):
            for rc in range(8):
                emit_tps_tile(1, mt, rc, vch1[(mt, rc)])
            emit_h_reduce(1, mt)
        emit_h_allreduce(1)
        nc.scalar.copy(hTab[1][:], hTa[1][:])

        # tail: o1 = relu(u + 2048*(M0@h0 + M1@h1)) = relu(2048*z1t + w)
        z1ps = pw.tile([128, 4], F32, name="z1ps", tag="wps")
        for jt in range(4):
            for kb in range(4):
                nc.tensor.matmul(
                    z1ps[:, jt:jt + 1],
                    m1Ts[:, kb * MID + jt * 128: kb * MID + (jt + 1) * 128],
                    hTab[1][:, kb:kb + 1], start=(kb == 0), stop=(kb == 3))
        o1Tb = pp.tile([128, 4], BF16, name="o1Tb", tag="o1Tb")
        for jt in range(4):
            nc.scalar.activation(o1Tb[:, jt:jt + 1], z1ps[:, jt:jt + 1],
                                 AT.Relu, bias=w_sb[:, jt:jt + 1],
                                 scale=float(NOBJ))

        # fc2: out = relu(o1 @ fc2T + fc2b)   [1, 1024]
        out_sb = pp.tile([1, FINAL], F32, name="out_sb", tag="out_sb")
        for half in range(2):
            ops_ = pw.tile([1, 512], F32, name=f"ops{half}", tag="wps")
            for kb in range(4):
                nc.tensor.matmul(
                    ops_[:], o1Tb[:, kb:kb + 1],
                    fc2Ts[:, kb * FINAL + half * 512: kb * FINAL + half * 512 + 512],
                    start=(kb == 0), stop=False)
            nc.tensor.matmul(
                ops_[:], ones1[:],
                fc2bs[0:1, half * 512:(half + 1) * 512],
                start=False, stop=True)
            nc.scalar.activation(out_sb[0:1, half * 512:(half + 1) * 512],
                                 ops_[:], AT.Relu)
        nc.sync.dma_start(d["d_out"][:, :], out_sb[:])


def _prep_inputs(entities, relations, graph, obj_tab, head_tab, tail_tab, pred_tab,
                 lin_v_v, lin_v_g, lin_v_b, lin_q_v, lin_q_g, lin_q_b,
                 lin_a_v, lin_a_g, lin_a_b, fc1_w, fc1_b, fc2_w, fc2_b):
    ent = np.asarray(entities).astype(np.int64)
    rel = np.asarray(relations).astype(np.int64)
    graph = np.asarray(graph, dtype=np.float32)
    obj_tab = np.asarray(obj_tab, np.float32)
    head_tab = np.asarray(head_tab, np.float32)
    tail_tab = np.asarray(tail_tab, np.float32)
    pred_tab = np.asarray(pred_tab, np.float32)

    fc1_w = np.asarray(fc1_w, np.float32)
    fc1_b = np.asarray(fc1_b, np.float32)

    abc = np.zeros((GLIMPSES, 256, MID), np.float32)
    qw = np.zeros((GLIMPSES, 256, MID), np.float32)
    Wa = [None, None]
    ba = [None, None]
    for g in range(GLIMPSES):
        Wv = _wn(np.asarray(lin_v_v[g], np.float32), float(lin_v_g[g]))
        abc[g, 0:51] = head_tab[:51] @ Wv[:, 0:EMBED].T + np.asarray(lin_v_b[g], np.float32)
        abc[g, 51:102] = tail_tab[:51] @ Wv[:, EMBED:2 * EMBED].T
        abc[g, 102:153] = pred_tab[:51] @ Wv[:, 2 * EMBED:3 * EMBED].T
        Wq = _wn(np.asarray(lin_q_v[g], np.float32), float(lin_q_g[g]))
        qw[g, 0:151] = obj_tab @ Wq.T
        if g == 0:
            qw[0, 0:151] += np.asarray(lin_q_b[0], np.float32)
        Wa[g] = _wn(np.asarray(lin_a_v[g], np.float32), float(lin_a_g[g]))
        ba[g] = np.asarray(lin_a_b[g], np.float32)

    Wq1 = _wn(np.asarray(lin_q_v[1], np.float32), float(lin_q_g[1]))
    # z1bq*QSCALE = h0 @ (QSCALE*(Wq1 @ Wa0)).T + QSCALE*(ba0 @ Wq1.T + bq1)
    k0T = np.ascontiguousarray((Wq1 @ Wa[0]).T) * QSCALE
    kb0 = (ba[0] @ Wq1.T + np.asarray(lin_q_b[1], np.float32)).reshape(1, MID) * QSCALE
    # fc1 @ sg = u + 2048*(M0 @ h0 + M1 @ h1)
    m0T = np.ascontiguousarray((fc1_w @ Wa[0]).T)
    m1T = np.ascontiguousarray((fc1_w @ Wa[1]).T)

    oht = np.zeros((NCORES, 256, RCH), NP_FP8)
    ar = np.arange(RCH)
    for c in range(NCORES):
        rc = rel[c * RCH:(c + 1) * RCH]
        m = np.zeros((256, RCH), np.float32)
        m[rc[:, 0], ar] = 1.0
        m[rc[:, 1] + 51, ar] = 1.0
        m[rc[:, 2] + 102, ar] = 1.0
        oht[c] = m.astype(NP_FP8)

    ohet = np.zeros((256, NOBJ), np.float32)
    ohet[ent, np.arange(NOBJ)] = 1.0
    # ones row at cat 160 (partition 32 of k-subtile 1, ACT-writable):
    # broadcasts the z1bq correction to every object in glimpse 1
    ohet[160, :] = 1.0

    colsum = graph.sum(axis=1, dtype=np.float32) + 1e-9
    recipC = (CSCALE / (colsum * QSCALE)).reshape(NOB, 128).T.copy()

    cnt = np.bincount(ent, minlength=151).astype(np.float32)
    sgq0 = cnt @ obj_tab                       # column sums of q0  [512]
    u = (fc1_w @ sgq0 + float(NOBJ) * (fc1_w @ (ba[0] + ba[1])) + fc1_b)
    u = u.reshape(4, 128).T.copy()

    base = {
        "oht": None,  # per-core
        "abc": (abc * VSCALE).astype(NP_FP8),
        "ohet": ohet.astype(NP_FP8),
        "qw": (qw * QSCALE).astype(NP_FP8),
        "k0T": k0T.astype(NP_BF16),
        "kb0": kb0,
        "m0T": m0T.astype(NP_BF16),
        "m1T": m1T.astype(NP_BF16),
        "u": u,
        "fc2T": np.ascontiguousarray(fc2_w.astype(np.float32).T).astype(NP_BF16),
        "fc2b": np.asarray(fc2_b, np.float32).reshape(1, FINAL).astype(NP_BF16),
        "recipC": recipC,
    }
    in_maps = []
    for c in range(NCORES):
        m = dict(base)
        m["g8"] = np.ascontiguousarray(graph[:, c * RCH:(c + 1) * RCH]).astype(NP_FP8)
        m["oht"] = oht[c]
        in_maps.append(m)
    return in_maps


def kernel(**inputs):
    if "nc" not in _CACHE:
        _CACHE["nc"], _CACHE["in_names"] = _build()
    nc = _CACHE["nc"]
    in_maps = _prep_inputs(**inputs)
    res = bass_utils.run_bass_kernel_spmd(nc, in_maps, core_ids=list(range(NCORES)))
    return np.asarray(res.results[0]["out"], np.float32)
